# revision 1
# baseline (speedup 1.0000x reference)
"""PoseConsistencyLoss Trainium2 kernel (8-core SPMD Bass/Tile).

Math: the reference's outputs (loss, num_matches, mean_distance) depend only on
the per-landmark min squared distance over all splats:
  - matched = splat_positions[argmin] makes sum(sqerr) == min_dist^2 exactly,
  - so loss = sum(valid*minsq)/max(3*num,1), mean = sum(valid*sqrt(minsq))/max(num,1),
    num = sum(minsq < 1.0).
Sharding: splats split across 8 cores (8192 each); each core computes partial
column-mins of the [8192 x 2048] distance matrix, AllGather + local min, then a
replicated masked reduction. Output taken from core 0.

Distance matrix via a K=9 feature matmul on the PE:
  L = [-2*cx,-2*cy,-2*cz, cx^2,cy^2,cz^2, 1,1,1]  (landmarks, camera frame)
  S = [ sx,  sy,  sz,  1,  1,  1,  sx^2,sy^2,sz^2] (splats)
  D2[m,n] = sum_k L[k,m]*S[k,n]
fp32 matmul is 4 cy/row on TRN2; fp32r is 1 cy/row but rounds operands to 12-bit
mantissa. We recover ~fp32 accuracy with a hi/lo split (2 matmuls):
  D2 = L_hi*S_hi  +  (L_lo*S_hi + L_hi*S_lo)   [K=9 and K=18, PSUM-accumulated]
where hi = bitwise-truncate to 11 explicit mantissa bits (exact in fp32r).

Column-min consumes PSUM with a DVE/ACT split: some spans reduced directly on
the DVE (fp32 reduce_min), others copied PSUM->SBUF by the Scalar engine and
paired into tensor_tensor_reduce(min,min) ops that consume 2 streams/cycle.
"""

import os
import sys
import time

sys.path.insert(0, "/opt/trn_rl_repo")

import numpy as np

import concourse.bass as bass
import concourse.bacc as bacc
import concourse.tile as tile
from concourse import mybir
from concourse.bass_utils import run_bass_kernel_spmd

# Disk-cache NEFF compiles (neuronxcc is ~15 min/invocation on this 1-CPU box).
import concourse.bass_utils as _bu
import concourse.bass2jax as _b2j

_orig_compile_bir = _bu.compile_bir_kernel
_NEFF_CACHE = os.environ.get("BASS_NEFF_CACHE_DIR", "/tmp/bass_neff_cache")


def _cached_compile_bir(bir_json, tmpdir, neff_name="file.neff"):
    import hashlib
    import shutil

    h = hashlib.sha256(bir_json).hexdigest()[:24]
    os.makedirs(_NEFF_CACHE, exist_ok=True)
    cpath = os.path.join(_NEFF_CACHE, f"{h}_{neff_name}")
    out = os.path.join(tmpdir, neff_name)
    if os.path.exists(cpath):
        shutil.copyfile(cpath, out)
        return out
    p = _orig_compile_bir(bir_json, tmpdir, neff_name=neff_name)
    try:
        shutil.copyfile(p, cpath)
    except OSError:
        pass
    return p


_bu.compile_bir_kernel = _cached_compile_bir
_b2j.compile_bir_kernel = _cached_compile_bir

F32 = mybir.dt.float32
F32R = mybir.dt.float32r
U32 = mybir.dt.uint32
I32 = mybir.dt.int32
AF = mybir.ActivationFunctionType
ALU = mybir.AluOpType
AX = mybir.AxisListType

HI_MASK = 0xFFFFF000  # keep sign+exp+11 mantissa bits (fp32r-exact)
BIG = 3.0e38

FULL_CFG = dict(
    n_cores=8,
    s_per_core=8192,   # splats per core
    m_total=2048,      # landmarks
    span=1024,         # psum span (free elems, 2 banks)
    # per-mt span roles, cycled: D=direct DVE reduce, A=ACT copy to SBUF,
    # T=tensor_tensor_reduce pairing the previous A's copy with its own psum.
    roles=("D",),          # direct DVE reduce only (safest instruction mix)
    use_f32r_split=False,  # False -> plain fp32 matmul (4 cy/row, no split)
    use_collective=False,  # collectives hang on this axon setup; host-side min
)


def _roles_for(n_spans, pattern):
    """Assign a role to each span; every T must be preceded by an unpaired A."""
    roles = []
    pend_a = 0
    for i in range(n_spans):
        r = pattern[i % len(pattern)]
        if r == "T" and pend_a == 0:
            r = "D"
        if r == "A":
            # an A must have a following T; if this is the last span, direct it
            if i == n_spans - 1:
                r = "D"
        if r == "A":
            pend_a += 1
        if r == "T":
            pend_a -= 1
        roles.append(r)
    # orphan A at the end shouldn't happen due to check above, but make sure
    assert pend_a == 0, roles
    return roles


def build(cfg):
    """Build the SPMD Bass program. Returns (nc, input_names)."""
    C = cfg["n_cores"]
    S = cfg["s_per_core"]
    M = cfg["m_total"]
    SPAN = cfg["span"]
    MMSZ = 512  # matmul moving free dim
    assert SPAN % MMSZ == 0 and S % SPAN == 0 and M % 128 == 0
    MT = M // 128
    NSPAN = S // SPAN
    LM_F = M // 128  # free elems per partition in [128, *] landmark layout
    roles = _roles_for(NSPAN, cfg["roles"])
    split = cfg["use_f32r_split"]

    nc = bacc.Bacc(
        "TRN2", target_bir_lowering=False, debug=False, num_devices=C
    )

    # ---- I/O ----
    spT_d = nc.dram_tensor("spT", [3, S], F32, kind="ExternalInput")
    lmT_d = nc.dram_tensor("lmT", [3, M], F32, kind="ExternalInput")
    poseT_d = nc.dram_tensor("poseT", [4, 4], F32, kind="ExternalInput")
    mmdt = F32R if split else F32  # dtype of matmul-feeding tensors
    konst_d = nc.dram_tensor("konst", [6, S], mmdt, kind="ExternalInput")  # ones/zeros
    onec_d = nc.dram_tensor("ones_col", [128, 1], F32, kind="ExternalInput")
    use_cc = cfg.get("use_collective", True)
    if use_cc:
        loss_d = nc.dram_tensor("loss", [1], F32, kind="ExternalOutput")
        nmat_d = nc.dram_tensor("nmatch", [1], I32, kind="ExternalOutput")
        mean_d = nc.dram_tensor("meand", [1], F32, kind="ExternalOutput")
    else:
        part_out_d = nc.dram_tensor("partial", [M], F32, kind="ExternalOutput")

    # round-robin issuing engines for setup DMAs -> parallel DGE queues
    _dmaq = [nc.sync, nc.gpsimd, nc.scalar]
    _dmaqi = [0]

    def dq():
        e = _dmaq[_dmaqi[0] % len(_dmaq)]
        _dmaqi[0] += 1
        return e

    with tile.TileContext(nc) as tc:
        with (
            tc.tile_pool(name="persist", bufs=1) as persist,
            tc.tile_pool(name="setup", bufs=1) as setup,
            tc.tile_pool(name="stream", bufs=3) as stream,
            tc.tile_pool(name="dram", bufs=1, space="DRAM") as dp,
        ):
            # ================= landmark features =================
            KX = 15 if split else 9
            # Engine ops need 32-aligned start partitions, so all compute stays
            # at partition base 0; cross-partition placement is done by matmul
            # outputs and DMAs only.
            #   P1 = [c, -2c, 0]   P2 = [c, 1, 0]   (both [9, M] via hom matmul)
            #   L_f32 = P1 * P2 = [c^2, -2c, 0];  rows 6-8 ones via DMA
            #   hi = f32r(L); lo = L - hi
            pt = setup.tile([4, 4], F32)
            nc.sync.dma_start(pt[:], poseT_d[:])
            lhsA = setup.tile([4, 9], F32)
            nc.vector.memset(lhsA[:], 0.0)
            nc.vector.tensor_copy(lhsA[:, 0:3], pt[:, 0:3])
            nc.vector.tensor_scalar(lhsA[:, 3:6], pt[:, 0:3], -2.0, None, ALU.mult)
            lhsB = setup.tile([4, 9], F32)
            nc.vector.memset(lhsB[:], 0.0)
            nc.vector.tensor_copy(lhsB[:, 0:3], pt[:, 0:3])
            # e3 columns (select hom ones-row): [1,1,1] into row 3, cols 3-5
            dq().dma_start(lhsB[3:4, 3:6], konst_d[0:1, 0:3].bitcast(F32))
            homT = setup.tile([4, M], F32)
            nc.sync.dma_start(homT[0:3, :], lmT_d[:])
            nc.sync.dma_start(homT[3:4, :], konst_d[0:1, 0:M].bitcast(F32))

            feat_lm_hi = persist.tile([9, M], mmdt)
            if split:
                feat_lm_x = persist.tile([KX, M], mmdt)

            with tc.tile_pool(name="lmpsum", bufs=1, space="PSUM") as lpp:
                p1 = lpp.tile([9, M], F32)
                p2 = lpp.tile([9, M], F32)
                lmb = min(MMSZ, M)
                for b in range(M // lmb):
                    sl = slice(b * lmb, (b + 1) * lmb)
                    nc.tensor.matmul(
                        p1[:, sl], lhsA[:], homT[:, sl], start=True, stop=True
                    )
                    nc.tensor.matmul(
                        p2[:, sl], lhsB[:], homT[:, sl], start=True, stop=True
                    )
                p2s = setup.tile([9, M], F32)
                nc.scalar.copy(p2s[:], p2[:])
                lmf = setup.tile([9, M], F32)
                nc.vector.tensor_mul(lmf[:], p1[:], p2s[:])
            nc.vector.tensor_copy(feat_lm_hi[0:6, :], lmf[0:6, :])
            dq().dma_start(feat_lm_hi[6:9, :], konst_d[0:3, 0:M])  # ones
            if split:
                nc.vector.tensor_sub(
                    feat_lm_x[0:6, :], lmf[0:6, :], feat_lm_hi[0:6, :].bitcast(F32)
                )
                dq().dma_start(feat_lm_x[6:9, :], konst_d[3:6, 0:M])  # zeros(L_lo 1s)
                # rows 9-11 pair with S c_lo -> L_hi(-2c); rows 12-14 pair with
                # S sq_lo -> L_hi ones
                dq().dma_start(feat_lm_x[9:12, :], feat_lm_hi[3:6, :])
                dq().dma_start(feat_lm_x[12:15, :], feat_lm_hi[6:9, :])

            # ================= splat features =================
            # feat_sp rows: 0-2 ones, 3-5 c_hi, 6-8 sq_hi, 9-11 c_lo, 12-14 sq_lo
            feat_sp = persist.tile([KX, S], mmdt)
            # nat layout [P, 256] of the flat [3*S] stream; 256 divides S so
            # DMA inner runs between [*,256] and [3, S] shapes stay commensurable
            natw = 2048
            natp = S * 3 // natw
            assert natp <= 128 and S % natw == 0
            nat = setup.tile([natp, natw], F32)
            nc.sync.dma_start(
                nat[:],
                spT_d[:].rearrange("a b -> (a b)").rearrange("(p f) -> p f", p=natp),
            )
            nat_sq = setup.tile([natp, natw], F32)
            nc.scalar.activation(nat_sq[:], nat[:], AF.Square)
            if split:
                # hi = fp32r round-on-write (matches PE operand rounding);
                # lo = x - hi, itself rounded to fp32r on write (error ~2^-24|x|)
                nat_hi = setup.tile([natp, natw], F32R)
                nc.vector.tensor_copy(nat_hi[:], nat[:])
                nat_lo = setup.tile([natp, natw], F32R)
                nc.vector.tensor_sub(nat_lo[:], nat[:], nat_hi[:].bitcast(F32))
                sq_hi = setup.tile([natp, natw], F32R)
                nc.vector.tensor_copy(sq_hi[:], nat_sq[:])
                sq_lo = setup.tile([natp, natw], F32R)
                nc.vector.tensor_sub(sq_lo[:], nat_sq[:], sq_hi[:].bitcast(F32))
            else:
                nat_hi, sq_hi = nat, nat_sq

            def row3(dst_rows, src):
                """DMA a nat-layout tile into 3 feature rows.

                Flat element order matches on both sides (partition-major)."""
                dq().dma_start(feat_sp[dst_rows : dst_rows + 3, :], src[:])

            dq().dma_start(feat_sp[0:3, :], konst_d[0:3, :])  # splat ones rows
            row3(3, nat_hi)
            row3(6, sq_hi)
            if split:
                row3(9, nat_lo)
                row3(12, sq_lo)

            # ================= main loop =================
            pp = tc.alloc_tile_pool(name="psum", bufs=4, space="PSUM")
            minsq = persist.tile([128, MT], F32)
            n_chain = roles.count("T")
            n_direct = roles.count("D")
            for mt in range(MT):
                lhs1 = feat_lm_hi[:, mt * 128 : (mt + 1) * 128]
                if split:
                    lhs2 = feat_lm_x[:, mt * 128 : (mt + 1) * 128]
                cols = setup.tile([128, n_direct + 1], F32, tag="mtcols")
                if n_chain:
                    chain = setup.tile([128, n_chain], F32, tag="mtchain")
                    scratch = setup.tile([128, SPAN], F32, tag="ttr_scratch")
                di = 0
                ti = 0
                last_a = None
                for si in range(NSPAN):
                    ps = pp.tile([128, SPAN], F32, tag="ps")
                    for h in range(SPAN // MMSZ):
                        off = si * SPAN + h * MMSZ
                        rhs1 = feat_sp[0:9, off : off + MMSZ]
                        pslice = ps[:, h * MMSZ : (h + 1) * MMSZ]
                        if split:
                            nc.tensor.matmul(
                                pslice, lhs1, rhs1, start=True, stop=False
                            )
                            rhs2 = feat_sp[0:KX, off : off + MMSZ]
                            nc.tensor.matmul(
                                pslice, lhs2, rhs2, start=False, stop=True
                            )
                        else:
                            nc.tensor.matmul(
                                pslice,
                                feat_lm_hi[:, mt * 128 : (mt + 1) * 128],
                                feat_sp[0:9, off : off + MMSZ],
                                start=True,
                                stop=True,
                            )
                    r = roles[si]
                    if r == "D":
                        nc.vector.tensor_reduce(
                            cols[:, di : di + 1], ps[:], AX.X, ALU.min
                        )
                        di += 1
                    elif r == "A":
                        sc = stream.tile([128, SPAN], F32, tag="actcopy")
                        nc.scalar.activation(sc[:], ps[:], AF.Copy)
                        last_a = sc
                    else:  # T
                        init = BIG if ti == 0 else chain[:, ti - 1 : ti]
                        nc.vector.tensor_tensor_reduce(
                            out=scratch[:],
                            in0=ps[:],
                            in1=last_a[:],
                            scale=1.0,
                            scalar=init,
                            op0=ALU.min,
                            op1=ALU.min,
                            accum_out=chain[:, ti : ti + 1],
                        )
                        ti += 1
                if ti > 0:
                    nc.vector.tensor_copy(cols[:, di : di + 1], chain[:, ti - 1 : ti])
                    di += 1
                nc.vector.tensor_reduce(
                    minsq[:, mt : mt + 1], cols[:, 0:di], AX.X, ALU.min
                )

            pp.release()

            if not use_cc:
                # per-core partial min out; global min + masked loss on host
                nc.sync.dma_start(
                    part_out_d[:].rearrange("(p f) -> p f", p=128), minsq[:]
                )
            else:
                    # ================= cross-core AllGather + min =================
                # two half-gathers: the first overlaps the second half of the loop
                MH = MT // 2
                HM = 128 * MH
                ag_outs = []
                for half in range(2):
                    part_d = dp.tile([HM], F32, tag=f"part{half}", name=f"part{half}")
                    ag_d = dp.tile(
                        [C * HM],
                        F32,
                        addr_space="Shared" if C > 4 else "Local",
                        tag=f"ag{half}",
                        name=f"ag{half}",
                    )
                    nc.sync.dma_start(
                        part_d[:].rearrange("(p f) -> p f", p=128),
                        minsq[:, half * MH : (half + 1) * MH],
                    )
                    nc.gpsimd.collective_compute(
                        "AllGather",
                        ALU.bypass,
                        replica_groups=[list(range(C))],
                        ins=[part_d[:]],
                        outs=[ag_d[:]],
                    )
                    ag_outs.append(ag_d)
                g = setup.tile([128, C * MT], F32)
                for half, ag_d in enumerate(ag_outs):
                    nc.sync.dma_start(
                        g[:, half * C * MH : (half + 1) * C * MH],
                        ag_d[:].rearrange("(r p f) -> p r f", r=C, p=128),
                    )
                gm = setup.tile([128, MT], F32)
                nc.vector.tensor_reduce(
                    gm[:].rearrange("p (h f) -> p h f", h=2),
                    g[:].rearrange("p (h r f) -> p h f r", h=2, r=C),
                    AX.X,
                    ALU.min,
                )

                # ================= replicated masked reduction =================
                msq = setup.tile([128, MT], F32)
                nc.vector.tensor_scalar(msq[:], gm[:], 0.0, None, ALU.max)
                d0 = setup.tile([128, MT], F32)
                nc.scalar.activation(d0[:], msq[:], AF.Sqrt)
                # one Newton step: d = 0.5*(d0 + msq/max(d0,eps))
                d0m = setup.tile([128, MT], F32)
                nc.vector.tensor_scalar(d0m[:], d0[:], 1e-20, None, ALU.max)
                rc = setup.tile([128, MT], F32)
                nc.vector.reciprocal(rc[:], d0m[:])
                dn = setup.tile([128, MT], F32)
                nc.vector.tensor_mul(dn[:], msq[:], rc[:])
                dd = setup.tile([128, MT], F32)
                nc.vector.tensor_add(dd[:], dn[:], d0[:])
                nc.vector.tensor_scalar(dd[:], dd[:], 0.5, None, ALU.mult)

                valid = setup.tile([128, MT], F32)
                nc.vector.tensor_scalar(valid[:], msq[:], 1.0, None, ALU.is_lt)
                vd = setup.tile([128, MT], F32)
                nc.vector.tensor_mul(vd[:], valid[:], dd[:])
                vsq = setup.tile([128, MT], F32)
                nc.vector.tensor_mul(vsq[:], valid[:], msq[:])

                stats = setup.tile([128, 3], F32)
                nc.vector.tensor_reduce(stats[:, 0:1], valid[:], AX.X, ALU.add)
                nc.vector.tensor_reduce(stats[:, 1:2], vd[:], AX.X, ALU.add)
                nc.vector.tensor_reduce(stats[:, 2:3], vsq[:], AX.X, ALU.add)

                onec = setup.tile([128, 1], F32)
                nc.sync.dma_start(onec[:], onec_d[:])
                fpp = tc.alloc_tile_pool(name="finpsum", bufs=1, space="PSUM")
                fin = fpp.tile([1, 3], F32, tag="fin", bufs=1)
                nc.tensor.matmul(fin[:], onec[:], stats[:], start=True, stop=True)

                cnt = fin[0:1, 0:1]
                den3 = setup.tile([1, 1], F32, tag="den3")
                nc.vector.tensor_scalar(den3[:], cnt, 3.0, 1.0, ALU.mult, ALU.max)
                den1 = setup.tile([1, 1], F32, tag="den1")
                nc.vector.tensor_scalar(den1[:], cnt, 1.0, None, ALU.max)
                rd3 = setup.tile([1, 1], F32, tag="rd3")
                nc.vector.reciprocal(rd3[:], den3[:])
                rd1 = setup.tile([1, 1], F32, tag="rd1")
                nc.vector.reciprocal(rd1[:], den1[:])
                loss_t = setup.tile([1, 1], F32, tag="losst")
                nc.vector.tensor_mul(loss_t[:], fin[0:1, 2:3], rd3[:])
                mean_t = setup.tile([1, 1], F32, tag="meant")
                nc.vector.tensor_mul(mean_t[:], fin[0:1, 1:2], rd1[:])
                num_i = setup.tile([1, 1], I32, tag="numi")
                nc.vector.tensor_copy(num_i[:], cnt)

                nc.sync.dma_start(loss_d[:], loss_t[:])
                nc.sync.dma_start(nmat_d[:], num_i[:])
                nc.sync.dma_start(mean_d[:], mean_t[:])
                fpp.release()

    nc.compile()
    return nc


def make_in_maps(cfg, splat_positions, camera_pose, landmarks_3d):
    C = cfg["n_cores"]
    S = cfg["s_per_core"]
    M = cfg["m_total"]
    LM_F = M // 128
    sp = np.ascontiguousarray(np.asarray(splat_positions, np.float32))
    pose = np.asarray(camera_pose, np.float32)
    lm = np.asarray(landmarks_3d, np.float32)
    konst = np.concatenate(
        [np.ones((3, S), np.float32), np.zeros((3, S), np.float32)], axis=0
    )
    ones_col = np.ones((128, 1), np.float32)
    poseT = np.ascontiguousarray(pose.T)
    lmT = np.ascontiguousarray(lm.T)
    maps = []
    for c in range(C):
        shard = sp[c * S : (c + 1) * S]
        maps.append(
            {
                "spT": np.ascontiguousarray(shard.T),
                "lmT": lmT,
                "poseT": poseT,
                "konst": konst,
                "ones_col": ones_col,
            }
        )
    return maps


_COMPILED = None


def _get_compiled():
    global _COMPILED
    if _COMPILED is None:
        _COMPILED = build(FULL_CFG)
    return _COMPILED


def kernel(
    splat_positions,
    camera_pose,
    landmarks_3d,
    landmarks_2d=None,
    camera_intrinsics=None,
    **_unused,
):
    nc = _get_compiled()
    in_maps = make_in_maps(FULL_CFG, splat_positions, camera_pose, landmarks_3d)
    core_ids = list(range(FULL_CFG["n_cores"]))
    try:
        res = run_bass_kernel_spmd(nc, in_maps, core_ids)
    except Exception:
        # one retry -- a previous run can leave the device wedged
        time.sleep(5.0)
        res = run_bass_kernel_spmd(nc, in_maps, core_ids)
    if FULL_CFG.get("use_collective", True):
        r0 = res.results[0]
        loss = np.float32(r0["loss"].reshape(-1)[0])
        num = np.int32(r0["nmatch"].reshape(-1)[0])
        meand = np.float32(r0["meand"].reshape(-1)[0])
        return loss, num, meand
    # host-side cross-core min + masked reduction (2048 elements)
    parts = np.stack([r["partial"] for r in res.results], axis=0)
    msq = np.maximum(parts.min(axis=0), np.float32(0.0)).astype(np.float32)
    d = np.sqrt(msq)
    valid = d < np.float32(1.0)
    num = np.int32(valid.sum())
    loss = np.float32(
        (msq * valid).sum(dtype=np.float32)
        / max(np.float32(3.0) * np.float32(num), np.float32(1.0))
    )
    meand = np.float32(
        (d * valid).sum(dtype=np.float32)
        / max(np.float32(num), np.float32(1.0))
    )
    return loss, num, meand


if __name__ == "__main__":
    # smoke-test build only
    build(FULL_CFG)
    print("build ok")



# revision 8
# speedup vs baseline: 2.1088x; 2.1088x over previous
"""PoseConsistencyLoss Trainium2 kernel (8-core SPMD Bass/Tile).

Math: the reference's outputs (loss, num_matches, mean_distance) depend only on
the per-landmark min squared distance over all splats:
  - matched = splat_positions[argmin] makes sum(sqerr) == min_dist^2 exactly,
  - so loss = sum(valid*minsq)/max(3*num,1), mean = sum(valid*sqrt(minsq))/max(num,1),
    num = sum(minsq < 1.0).
Sharding: splats split across 8 cores (8192 each); each core computes partial
column-mins of the [8192 x 2048] distance matrix; host does the 8-way min and
the masked reduction over 2048 landmarks.

Device computes E[m,n] = -2 c_m . s_n + ||s_n||^2  (c = landmarks in camera
frame). ||c_m||^2 is a per-landmark (per-PSUM-row) constant, so it cannot
change the argmin over n -- the host adds it back after the cross-core min:
  minsq = max(min_cores min_n E + ||c||^2, 0).

E is ONE K=15 f32r matmul per [128 x 512] tile (PE cost is K-independent:
moving-free-size x 1 cy/row for f32r, vs 4 cy/row for fp32). f32r rounds
operands to 12-bit mantissa; full precision is recovered with hi/lo splits
folded into the K dim:
  k 0-2 : -2c_hi * s_hi      k 9-11 : 1 * (s^2)_hi
  k 3-5 : -2c_lo * s_hi      k 12-14: 1 * (s^2)_lo
  k 6-8 : -2c_hi * s_lo
(dropped lo*lo terms ~2^-24). Landmark features are built host-side (O(M)
work); splat features (square + hi/lo) are built on device.

Column-min consumes PSUM with a 3-engine split (DVE tensor_tensor_reduce
faults on this runtime, and GPSIMD cannot read PSUM): the first
`group` spans are copied PSUM->SBUF by the Scalar engine and chain-merged
(elementwise min) by the Pool engine in SBUF; the remaining spans are
reduced directly from PSUM by the DVE, which also reduces the merged span.
Only the DVE can reduce along the free axis, so the ACT+Pool pre-merge is
what keeps DVE work below the PE's matmul time.
"""

import os
import sys
import time

sys.path.insert(0, "/opt/trn_rl_repo")

import numpy as np

import concourse.bass as bass
import concourse.bacc as bacc
import concourse.tile as tile
from concourse import mybir
from concourse.bass_utils import run_bass_kernel_spmd

# Disk-cache NEFF compiles.
import concourse.bass_utils as _bu
import concourse.bass2jax as _b2j

_orig_compile_bir = _bu.compile_bir_kernel
_NEFF_CACHE = os.environ.get("BASS_NEFF_CACHE_DIR", "/tmp/bass_neff_cache")


def _cached_compile_bir(bir_json, tmpdir, neff_name="file.neff"):
    import hashlib
    import shutil

    h = hashlib.sha256(bir_json).hexdigest()[:24]
    os.makedirs(_NEFF_CACHE, exist_ok=True)
    cpath = os.path.join(_NEFF_CACHE, f"{h}_{neff_name}")
    out = os.path.join(tmpdir, neff_name)
    if os.path.exists(cpath):
        shutil.copyfile(cpath, out)
        return out
    p = _orig_compile_bir(bir_json, tmpdir, neff_name=neff_name)
    try:
        shutil.copyfile(p, cpath)
    except OSError:
        pass
    return p


_bu.compile_bir_kernel = _cached_compile_bir
_b2j.compile_bir_kernel = _cached_compile_bir

F32 = mybir.dt.float32
F32R = mybir.dt.float32r
I32 = mybir.dt.int32
AF = mybir.ActivationFunctionType
ALU = mybir.AluOpType
AX = mybir.AxisListType

HI_MASK = 0xFFFFF000  # keep sign+exp+11 mantissa bits (fp32r-exact)
BIG = 3.0e38

FULL_CFG = dict(
    n_cores=8,
    s_per_core=8192,   # splats per core
    m_total=2048,      # landmarks
    span=1024,         # psum span (free elems, 2 banks)
    psum_bufs=4,
    group=0,           # spans per mt pre-merged via ACT copy + Pool min
)

KX = 15  # feature rows


def build(cfg):
    """Build the SPMD Bass program."""
    C = cfg["n_cores"]
    S = cfg["s_per_core"]
    M = cfg["m_total"]
    SPAN = cfg["span"]
    MMSZ = 512  # matmul moving free dim (hw max)
    assert SPAN % MMSZ == 0 and S % SPAN == 0 and M % 128 == 0
    MT = M // 128
    NSPAN = S // SPAN
    G = cfg["group"]
    assert 0 <= G <= NSPAN and G != 1

    nc = bacc.Bacc(
        "TRN2", target_bir_lowering=False, debug=False, num_devices=C
    )

    # ---- I/O ----
    spT_d = nc.dram_tensor("spT", [3, S], F32, kind="ExternalInput")
    featlm_d = nc.dram_tensor("featlm", [KX, M], F32R, kind="ExternalInput")
    part_out_d = nc.dram_tensor("partial", [M], F32, kind="ExternalOutput")

    # round-robin issuing engines for setup DMAs -> parallel DGE queues
    _dmaq = [nc.sync, nc.gpsimd, nc.scalar]
    _dmaqi = [0]

    def dq():
        e = _dmaq[_dmaqi[0] % len(_dmaq)]
        _dmaqi[0] += 1
        return e

    with tile.TileContext(nc) as tc:
        with (
            tc.tile_pool(name="persist", bufs=1) as persist,
            tc.tile_pool(name="setup", bufs=1) as setup,
            tc.tile_pool(name="stream", bufs=max(3, G + 1)) as stream,
            tc.tile_pool(name="mstream", bufs=2) as mstream,
        ):
            # ================= landmark features (host-built) =================
            feat_lm = persist.tile([KX, M], F32R)
            nc.sync.dma_start(feat_lm[:], featlm_d[:])

            # ================= splat features =================
            # feat_sp rows: 0-2 s_hi, 3-5 s_hi(dup), 6-8 s_lo, 9-11 sq_hi,
            # 12-14 sq_lo
            feat_sp = persist.tile([KX, S], F32R)
            # nat layout [12, 2048] of the flat [3*S] stream; flat element
            # order matches the [3, S] feature rows (partition-major).
            natw = 2048
            natp = S * 3 // natw
            assert natp <= 128 and S % natw == 0
            nat = setup.tile([natp, natw], F32)
            nc.sync.dma_start(
                nat[:],
                spT_d[:].rearrange("a b -> (a b)").rearrange("(p f) -> p f", p=natp),
            )
            nat_sq = setup.tile([natp, natw], F32)
            nc.scalar.activation(nat_sq[:], nat[:], AF.Square)
            # hi = fp32r round-on-write (matches PE operand rounding);
            # lo = x - hi, itself rounded to fp32r on write (error ~2^-24|x|)
            nat_hi = setup.tile([natp, natw], F32R)
            nc.vector.tensor_copy(nat_hi[:], nat[:])
            nat_lo = setup.tile([natp, natw], F32R)
            nc.vector.tensor_sub(nat_lo[:], nat[:], nat_hi[:].bitcast(F32))
            sq_hi = setup.tile([natp, natw], F32R)
            nc.gpsimd.tensor_copy(sq_hi[:], nat_sq[:])
            sq_lo = setup.tile([natp, natw], F32R)
            nc.gpsimd.tensor_sub(sq_lo[:], nat_sq[:], sq_hi[:].bitcast(F32))

            def row3(dst_rows, src):
                """DMA a nat-layout tile into 3 feature rows (flat order
                matches on both sides, partition-major)."""
                dq().dma_start(feat_sp[dst_rows : dst_rows + 3, :], src[:])

            row3(0, nat_hi)
            row3(3, nat_hi)
            row3(6, nat_lo)
            row3(9, sq_hi)
            row3(12, sq_lo)

            # ================= main loop =================
            pp = tc.alloc_tile_pool(name="psum", bufs=cfg["psum_bufs"], space="PSUM")
            minsq = persist.tile([128, MT], F32)
            n_cols = (NSPAN - G) + (1 if G else 0)
            for mt in range(MT):
                lhs = feat_lm[:, mt * 128 : (mt + 1) * 128]
                cols = setup.tile([128, n_cols], F32, tag="mtcols")
                acopies = []
                di = 0
                for si in range(NSPAN):
                    ps = pp.tile([128, SPAN], F32, tag="ps")
                    for h in range(SPAN // MMSZ):
                        off = si * SPAN + h * MMSZ
                        nc.tensor.matmul(
                            ps[:, h * MMSZ : (h + 1) * MMSZ],
                            lhs,
                            feat_sp[:, off : off + MMSZ],
                            start=True,
                            stop=True,
                        )
                    if si < G:
                        sc = stream.tile([128, SPAN], F32, tag="actcopy")
                        nc.scalar.activation(sc[:], ps[:], AF.Copy)
                        acopies.append(sc)
                        if si == 1:
                            m = mstream.tile([128, SPAN], F32, tag="merge")
                            nc.gpsimd.tensor_tensor(
                                m[:], acopies[0][:], acopies[1][:], ALU.min
                            )
                        elif si > 1:
                            m2 = mstream.tile([128, SPAN], F32, tag="merge")
                            nc.gpsimd.tensor_tensor(m2[:], m[:], sc[:], ALU.min)
                            m = m2
                    else:
                        nc.vector.tensor_reduce(
                            cols[:, di : di + 1], ps[:], AX.X, ALU.min
                        )
                        di += 1
                if G:
                    nc.vector.tensor_reduce(
                        cols[:, di : di + 1], m[:], AX.X, ALU.min
                    )
                    di += 1
                nc.vector.tensor_reduce(
                    minsq[:, mt : mt + 1], cols[:, 0:di], AX.X, ALU.min
                )
            pp.release()

            # per-core partial min out; global min + masked loss on host
            nc.sync.dma_start(
                part_out_d[:].rearrange("(p f) -> p f", p=128), minsq[:]
            )

    nc.compile()
    return nc


def _f32r_trunc(x):
    return (np.ascontiguousarray(x, np.float32).view(np.uint32) & np.uint32(HI_MASK)).view(np.float32)


def _landmarks_cam(camera_pose, landmarks_3d):
    pose = np.asarray(camera_pose, np.float32)
    lm = np.asarray(landmarks_3d, np.float32)
    hom = np.concatenate([lm, np.ones((lm.shape[0], 1), np.float32)], axis=1)
    return (pose @ hom.T).T[:, :3].astype(np.float32)  # [M, 3]


def make_in_maps(cfg, splat_positions, camera_pose, landmarks_3d):
    C = cfg["n_cores"]
    S = cfg["s_per_core"]
    M = cfg["m_total"]
    sp = np.ascontiguousarray(np.asarray(splat_positions, np.float32))
    cam = _landmarks_cam(camera_pose, landmarks_3d)  # [M, 3]
    m2c = (-2.0 * cam).astype(np.float32)
    hi = _f32r_trunc(m2c)
    lo = _f32r_trunc(m2c - hi)
    featlm = np.empty((KX, M), np.float32)
    featlm[0:3] = hi.T
    featlm[3:6] = lo.T
    featlm[6:9] = hi.T
    featlm[9:15] = 1.0
    maps = []
    for c in range(C):
        shard = sp[c * S : (c + 1) * S]
        maps.append(
            {
                "spT": np.ascontiguousarray(shard.T),
                "featlm": featlm,
            }
        )
    return maps


_COMPILED = None


def _get_compiled():
    global _COMPILED
    if _COMPILED is None:
        _COMPILED = build(FULL_CFG)
    return _COMPILED


def kernel(
    splat_positions,
    camera_pose,
    landmarks_3d,
    landmarks_2d=None,
    camera_intrinsics=None,
    **_unused,
):
    nc = _get_compiled()
    in_maps = make_in_maps(FULL_CFG, splat_positions, camera_pose, landmarks_3d)
    core_ids = list(range(FULL_CFG["n_cores"]))
    try:
        res = run_bass_kernel_spmd(nc, in_maps, core_ids)
    except Exception:
        # one retry -- a previous run can leave the device wedged
        time.sleep(5.0)
        res = run_bass_kernel_spmd(nc, in_maps, core_ids)

    # host-side: cross-core min of E, add ||c||^2 back, masked reduction
    M = FULL_CFG["m_total"]
    MT = M // 128
    parts = np.stack([r["partial"] for r in res.results], axis=0)  # [C, M]
    emin = parts.min(axis=0)  # index m' = p*MT + mt
    cam = _landmarks_cam(camera_pose, landmarks_3d)
    csq = np.sum(cam.astype(np.float32) ** 2, axis=1, dtype=np.float32)  # [M]
    csq_aligned = np.ascontiguousarray(csq.reshape(MT, 128).T).reshape(-1)
    msq = np.maximum(emin + csq_aligned, np.float32(0.0)).astype(np.float32)
    d = np.sqrt(msq)
    valid = d < np.float32(1.0)
    num = np.int32(valid.sum())
    loss = np.float32(
        (msq * valid).sum(dtype=np.float32)
        / max(np.float32(3.0) * np.float32(num), np.float32(1.0))
    )
    meand = np.float32(
        (d * valid).sum(dtype=np.float32)
        / max(np.float32(num), np.float32(1.0))
    )
    return loss, num, meand


if __name__ == "__main__":
    # smoke-test build only
    build(FULL_CFG)
    print("build ok")
